# revision 17
# baseline (speedup 1.0000x reference)
"""Bahdanau-attention kernel for Trainium2 (Bass/Tile), data-parallel over batch.

Reference computation (per batch b, pixels p=196, enc dim e=2048, att dim a=512):
    att1 = enc @ W_enc                      [B, P, A]
    att2 = dec @ W_dec + b_dec              [B, A]
    s    = tanh(att1 + att2 + b_enc) @ W_full   (+ b_full, irrelevant to softmax)
    alpha = softmax(s, axis=pixels)
    ctx  = sum_p alpha[p] * enc[p]

Sharding: batch 512 -> 8 cores x 64 batches. Weights replicated.

Per-core kernel (single HBM pass over enc):
  - 64*196 = 12544 rows, processed as 98 tiles of 128 rows.
  - per tile: PE-transpose enc -> att1 matmuls (float32r, N=512) into PSUM;
    a "selection" matmul adds att2+biases broadcast per-row; tanh (ScalarE);
    fused mul+reduce (VectorE) for scores; exp (no max subtraction: |s| <=
    sum|W_full| ~ 11.6, exp can't overflow fp32); masked-alpha matmul
    accumulates unnormalized context into persistent PSUM [64, 2048].
  - end: Z = sum exp(s) per batch, alpha = e/Z, ctx = ctx_unnorm/Z.

The row->batch selection masks depend only on shapes; they are precomputed on
the host and shipped as constant inputs (sel_all for the att2 broadcast
matmul, selT_all for masking exp'd scores into per-batch alpha columns).
"""

import sys

for _p in ("/opt/trn_rl_repo",):
    if _p not in sys.path:
        sys.path.insert(0, _p)

from contextlib import ExitStack

import numpy as np

import concourse.bass as bass
import concourse.mybir as mybir
import concourse.tile as tile
from concourse import bacc
from concourse.bass import ds, ts
from concourse.masks import make_identity

FP32 = mybir.dt.float32
FP32R = mybir.dt.float32r

N_CORES = 8
BATCH = 512
NPIX = 196
ENC_DIM = 2048
DEC_DIM = 512
ATT_DIM = 512

B = BATCH // N_CORES          # 64 batches per core
ROWS = B * NPIX               # 12544 rows per core
P = 128                       # partition tile
NT = ROWS // P                # 98 row-tiles
KE = ENC_DIM // P             # 16 contraction blocks for att1
KD = DEC_DIM // P             # 4 contraction blocks for att2
NSL = ENC_DIM // 512          # 4 n-slices of 512 for ctx


def r32(ap):
    return ap.bitcast(FP32R)


def tile_seg(t):
    """(b0, r): tile t rows [0, r) belong to batch b0, rows [r, 128) to b0+1."""
    row0 = t * P
    b0 = row0 // NPIX
    r = min(P, (b0 + 1) * NPIX - row0)
    return b0, r


def make_sel_masks():
    """Compressed segment masks.

    sel2[j, t, m]  = 1.0 iff row 128t+m is in segment j of tile t
    selT2[m, t, j] = same, transposed layout
    """
    sel2 = np.zeros((2, NT, P), np.float32)
    selT2 = np.zeros((P, NT, 2), np.float32)
    for t in range(NT):
        b0, r = tile_seg(t)
        sel2[0, t, 0:r] = 1.0
        selT2[0:r, t, 0] = 1.0
        if r < P:
            sel2[1, t, r:P] = 1.0
            selT2[r:P, t, 1] = 1.0
    return sel2, selT2


def build_program(opts=None):
    o = {"enc_bufs": 4, "encT_bufs": 12, "h_bufs": 3, "scr_bufs": 3,
         "small_bufs": 4, "alt_bufs": 4, "tr_bufs": 4, "a1_bufs": 2,
         "a2p_bufs": 3,
         "no_ctx": False, "no_att1": False, "no_tr": False, "no_score": False,
         "dma_split": 1}
    if opts:
        o.update(opts)
    nc = bacc.Bacc("TRN2", debug=False, num_devices=N_CORES)

    enc_d = nc.dram_tensor("enc", [ROWS, ENC_DIM], FP32R, kind="ExternalInput")
    dec_d = nc.dram_tensor("dec", [B, DEC_DIM], FP32, kind="ExternalInput")
    wenc_d = nc.dram_tensor("w_enc", [ENC_DIM, ATT_DIM], FP32R, kind="ExternalInput")
    benc_d = nc.dram_tensor("b_enc", [1, ATT_DIM], FP32, kind="ExternalInput")
    wdec_d = nc.dram_tensor("w_dec", [DEC_DIM, ATT_DIM], FP32, kind="ExternalInput")
    bdec_d = nc.dram_tensor("b_dec", [1, ATT_DIM], FP32, kind="ExternalInput")
    wfull_d = nc.dram_tensor("w_full", [1, ATT_DIM], FP32, kind="ExternalInput")
    sel_d = nc.dram_tensor("sel2_all", [2, NT * P], FP32R, kind="ExternalInput")
    selT_d = nc.dram_tensor("selT2_all", [P, NT * 2], FP32R, kind="ExternalInput")
    att2b_dram = nc.dram_tensor("att2b_scratch", [B + 1, ATT_DIM], FP32R,
                                kind="Internal")

    ctx_d = nc.dram_tensor("ctx_out", [B, ENC_DIM], FP32, kind="ExternalOutput")
    alpha_d = nc.dram_tensor("alpha_out", [B, NPIX], FP32, kind="ExternalOutput")

    enc_ap = enc_d.ap()
    with tile.TileContext(nc) as tc, ExitStack() as ctx:
        consts = ctx.enter_context(tc.tile_pool(name="consts", bufs=1))

        ident = consts.tile([P, P], FP32)
        make_identity(nc, ident[:])
        ident_r = consts.tile([P, P], FP32R)
        nc.vector.tensor_copy(ident_r[:], ident[:])

        # W_enc block k holds rows [k*128,(k+1)*128), partition = e within block
        wenc_sb = consts.tile([P, KE, ATT_DIM], FP32R)
        nc.sync.dma_start(
            wenc_sb[:], wenc_d.ap().rearrange("(k p) a -> p k a", p=P)
        )
        wdec_sb = consts.tile([P, KD, ATT_DIM], FP32)
        nc.sync.dma_start(
            wdec_sb[:], wdec_d.ap().rearrange("(k p) a -> p k a", p=P)
        )

        selT2_sb = consts.tile([P, NT, 2], FP32R)
        nc.sync.dma_start(selT2_sb[:], selT_d.ap().rearrange("m (t j) -> m t j", t=NT))

        wfull_row = consts.tile([1, ATT_DIM], FP32)
        nc.sync.dma_start(wfull_row[:], wfull_d.ap())
        benc_row = consts.tile([1, ATT_DIM], FP32)
        nc.sync.dma_start(benc_row[:], benc_d.ap())
        bdec_row = consts.tile([1, ATT_DIM], FP32)
        nc.sync.dma_start(bdec_row[:], bdec_d.ap())
        bias_row = consts.tile([1, ATT_DIM], FP32)
        nc.vector.tensor_add(bias_row[:], benc_row[:], bdec_row[:])

        ones_row = consts.tile([1, P], FP32)
        nc.vector.memset(ones_row[:], 1.0)
        zero_row = consts.tile([1, 512], FP32)
        nc.vector.memset(zero_row[:], 0.0)
        zeros_pp = consts.tile([P, P], FP32)
        nc.vector.memset(zeros_pp[:], 0.0)
        zeros_pp_r = consts.tile([P, P], FP32R)
        nc.vector.tensor_copy(zeros_pp_r[:], zeros_pp[:])

        dec_sb = consts.tile([B, DEC_DIM], FP32)
        nc.sync.dma_start(dec_sb[:], dec_d.ap())

        # ---- setup: W_full broadcast [128, 512] and att2b [64, 512] ----
        wfull_sb = consts.tile([P, ATT_DIM], FP32)
        att2b_sb = consts.tile([B, ATT_DIM], FP32R)
        with tc.tile_pool(name="setup_psum", bufs=1, space="PSUM") as spsum:
            wf_ps = spsum.tile([P, ATT_DIM], FP32)
            nc.tensor.matmul(wf_ps[:], ones_row[:, 0:P], wfull_row[:],
                             start=True, stop=True)
            nc.vector.tensor_copy(wfull_sb[:], wf_ps[:])

            # decT: [64, 512] -> 4 x [128, 64]
            decT_sb = consts.tile([P, KD, B], FP32)
            for l in range(KD):
                dT_ps = spsum.tile([P, B], FP32)
                nc.tensor.transpose(dT_ps[:], dec_sb[:, ts(l, P)],
                                    ident[0:B, 0:B])
                nc.vector.tensor_copy(decT_sb[:, l, :], dT_ps[:])

            att2_ps = spsum.tile([B, ATT_DIM], FP32)
            for l in range(KD):
                nc.tensor.matmul(att2_ps[:], decT_sb[:, l, :], wdec_sb[:, l, :],
                                 start=(l == 0), stop=False)
            # += ones[64] x (b_enc + b_dec)
            nc.tensor.matmul(att2_ps[:], ones_row[:, 0:B], bias_row[:],
                             start=False, stop=True)
            nc.vector.tensor_copy(att2b_sb[:], att2_ps[:])
        nc.sync.dma_start(att2b_dram.ap()[0:B, :], att2b_sb[:])
        zrow_r = consts.tile([1, ATT_DIM], FP32R)
        nc.vector.tensor_copy(zrow_r[:], zero_row[:])
        nc.sync.dma_start(att2b_dram.ap()[B:B + 1, :], zrow_r[:])

        # ---- persistent context accumulator, packed [128, 1024] = 2 banks:
        # partition b + 64*h holds e-columns [1024h, 1024h+1024) of batch b
        ctx_ps = None
        if not o["no_ctx"]:
            ctx_psum_pool = ctx.enter_context(
                tc.tile_pool(name="ctx_psum", bufs=1, space="PSUM"))
            ctx_ps = ctx_psum_pool.tile([P, 1024], FP32)
            for n in range(2):
                nc.tensor.matmul(ctx_ps[:, ts(n, 512)], ones_row[:, 0:P],
                                 zero_row[:], start=True, stop=False,
                                 skip_group_check=True)

        # exp'd scores: column t = rows [128t, 128t+128)
        ea_all = consts.tile([P, NT], FP32)

        enc_pool = ctx.enter_context(tc.tile_pool(name="enc", bufs=o["enc_bufs"]))
        encT_pool = ctx.enter_context(tc.tile_pool(name="encT", bufs=o["encT_bufs"]))
        h_pool = ctx.enter_context(tc.tile_pool(name="h", bufs=o["h_bufs"]))
        scratch_pool = ctx.enter_context(tc.tile_pool(name="scratch", bufs=o["scr_bufs"]))
        small_pool = ctx.enter_context(tc.tile_pool(name="small", bufs=o["small_bufs"]))
        alt_pool = ctx.enter_context(tc.tile_pool(name="alt", bufs=o["alt_bufs"]))
        a2p_pool = ctx.enter_context(tc.tile_pool(name="a2p", bufs=o["a2p_bufs"]))
        sel2_pool = ctx.enter_context(tc.tile_pool(name="sel2", bufs=o["a2p_bufs"]))

        with tc.tile_pool(name="tr_psum", bufs=o["tr_bufs"], space="PSUM") as tr_psum, \
             tc.tile_pool(name="att1_psum", bufs=o["a1_bufs"], space="PSUM") as att1_psum:
            for t in range(NT):
                row0 = t * P

                enc_t = enc_pool.tile([P, ENC_DIM], FP32R, tag="enc")
                nsp = o["dma_split"]
                for si in range(nsp):
                    w = ENC_DIM // nsp
                    nc.sync.dma_start(enc_t[:, ts(si, w)],
                                      enc_ap[ds(row0, P), ts(si, w)])

                # transpose enc tile: 4 groups of 4x[128,128] -> encT [128,512]
                encT = []
                for g in range(0 if o["no_tr"] else 4):
                    tr_ps = tr_psum.tile([P, 512], FP32R, tag="tr")
                    for j in range(4):
                        nc.tensor.transpose(tr_ps[:, ts(j, P)],
                                            enc_t[:, ts(4 * g + j, P)], ident_r[:])
                    eT = encT_pool.tile([P, 512], FP32R, tag="encT")
                    if g % 2 == 0:
                        nc.vector.tensor_copy(eT[:], tr_ps[:])
                    else:
                        nc.scalar.copy(eT[:], tr_ps[:])
                    encT.append(eT)

                # att1 = encT.T @ W_enc blocks, accumulated over 16 k-blocks
                a1_ps = att1_psum.tile([P, ATT_DIM], FP32, tag="att1")
                if o["no_att1"] or o["no_tr"]:
                    nc.tensor.matmul(a1_ps[:], att2b_sb[0:2, :],
                                     att2b_sb[0:2, :], start=True, stop=True)
                for k in range(0 if (o["no_att1"] or o["no_tr"]) else KE):
                    g, j = divmod(k, 4)
                    nc.tensor.matmul(a1_ps[:], encT[g][:, ts(j, P)],
                                     wenc_sb[:, k, :],
                                     start=(k == 0), stop=False)

                # += sel.T @ att2b : adds att2b[row's batch] to every row
                if not (o["no_att1"] or o["no_tr"]):
                    b0, r = tile_seg(t)
                    a2p = a2p_pool.tile([2, ATT_DIM], FP32R, tag="a2p")
                    nc.sync.dma_start(a2p[:], att2b_dram.ap()[ds(b0, 2), :])
                    s2t = sel2_pool.tile([2, P], FP32R, tag="s2t")
                    nc.sync.dma_start(
                        s2t[:], sel_d.ap().rearrange("j (t m) -> j t m", t=NT)[:, t, :])
                    nc.tensor.matmul(a1_ps[:], s2t[:],
                                     a2p[:], start=False, stop=True)

                # h = tanh(att1), scores = h . W_full
                h = h_pool.tile([P, ATT_DIM], FP32, tag="h")
                nc.scalar.activation(h[:], a1_ps[:],
                                     mybir.ActivationFunctionType.Tanh)
                sc = small_pool.tile([P, 1], FP32, tag="sc")
                if o["no_score"]:
                    nc.vector.tensor_copy(sc[:], h[:, 0:1])
                else:
                    scr = scratch_pool.tile([P, ATT_DIM], FP32, tag="scr")
                    nc.vector.tensor_mul(scr[:], h[:], wfull_sb[:])
                    nc.vector.tensor_reduce(sc[:], scr[:],
                                            axis=mybir.AxisListType.X,
                                            op=mybir.AluOpType.add)

                # ea = exp(scores), collected as column t
                nc.scalar.activation(ea_all[:, ds(t, 1)], sc[:],
                                     mybir.ActivationFunctionType.Exp)

                # alT[:, b] = ea masked to batch b's rows; ctx += alT.T @ enc
                b0t, rt = tile_seg(t)
                ncols = 2 if (rt < P and b0t + 1 < B) else 1
                alTh = []
                for hh in range(2):
                    a = alt_pool.tile([P, P], FP32R, tag=f"alT{hh}")
                    nc.vector.tensor_copy(a[:], zeros_pp_r[:])
                    nc.vector.tensor_scalar_mul(
                        a[:, ds(b0t + 64 * hh, ncols)],
                        selT2_sb[:, t, 0:ncols], ea_all[:, ds(t, 1)])
                    alTh.append(a)
                for n in range(0 if o["no_ctx"] else NSL):
                    nc.tensor.matmul(ctx_ps[:, ts(n % 2, 512)], alTh[n // 2][:],
                                     enc_t[:, ts(n, 512)],
                                     start=False,
                                     stop=(t == NT - 1 and n == NSL - 1),
                                     skip_group_check=True)

        # ---- finalize ----
        e_sb = consts.tile([B, NPIX], FP32)
        with tc.tile_pool(name="fin_psum", bufs=1, space="PSUM") as fpsum:
            eaT_ps = fpsum.tile([NT, P], FP32)
            nc.tensor.transpose(eaT_ps[:], ea_all[:], ident[:])
            eaT_sb = consts.tile([NT, P], FP32)
            nc.vector.tensor_copy(eaT_sb[:], eaT_ps[:])
        # flat row order [NT,128] -> [64,196] reshape via a DRAM bounce
        ea_dram = nc.dram_tensor("ea_scratch", [NT, P], FP32, kind="Internal")
        nc.sync.dma_start(ea_dram.ap(), eaT_sb[:])
        nc.sync.dma_start(e_sb[:],
                          ea_dram.ap().rearrange("t m -> (t m)")
                          .rearrange("(b p) -> b p", b=B))

        z = consts.tile([B, 1], FP32)
        nc.vector.tensor_reduce(z[:], e_sb[:], axis=mybir.AxisListType.X,
                                op=mybir.AluOpType.add)
        rz = consts.tile([B, 1], FP32)
        nc.vector.reciprocal(rz[:], z[:])

        alpha_sb = consts.tile([B, NPIX], FP32)
        nc.vector.tensor_scalar_mul(alpha_sb[:], e_sb[:], rz[:])
        nc.sync.dma_start(alpha_d.ap(), alpha_sb[:])

        ctx_sb = consts.tile([P, 1024], FP32)
        if o["no_ctx"]:
            nc.vector.memset(ctx_sb[:], 0.0)
        else:
            # dup[b, c] = 1 iff c == b or c == b + 64; z2 = dup.T @ z
            dup_sb = consts.tile([B, P], FP32)
            nc.gpsimd.memset(dup_sb[:], 0.0)
            nc.gpsimd.affine_select(
                out=dup_sb[:], in_=dup_sb[:],
                compare_op=mybir.AluOpType.not_equal, fill=1.0, base=0,
                pattern=[[-1, P]], channel_multiplier=1)
            nc.gpsimd.affine_select(
                out=dup_sb[:], in_=dup_sb[:],
                compare_op=mybir.AluOpType.not_equal, fill=1.0, base=64,
                pattern=[[-1, P]], channel_multiplier=1)
            with tc.tile_pool(name="z2_psum", bufs=1, space="PSUM") as zpool:
                z2_ps = zpool.tile([P, 1], FP32)
                nc.tensor.matmul(z2_ps[:], dup_sb[:], z[:],
                                 start=True, stop=True)
                rz2 = consts.tile([P, 1], FP32)
                nc.vector.reciprocal(rz2[:], z2_ps[:])
            nc.vector.tensor_scalar_mul(ctx_sb[:, 0:512], ctx_ps[:, 0:512],
                                        rz2[:])
            nc.scalar.activation(ctx_sb[:, 512:1024], ctx_ps[:, 512:1024],
                                 mybir.ActivationFunctionType.Copy, scale=rz2[:])
        nc.sync.dma_start(ctx_d.ap()[:, 0:1024], ctx_sb[0:B, :])
        nc.sync.dma_start(ctx_d.ap()[:, 1024:2048], ctx_sb[B:P, :])

    nc.compile()
    return nc


_NC_CACHE = None


def _get_nc():
    global _NC_CACHE
    if _NC_CACHE is None:
        _NC_CACHE = build_program()
    return _NC_CACHE


LAST_RESULTS = None


def kernel(encoder_out, decoder_hidden, W_enc, b_enc, W_dec, b_dec, W_full,
           b_full=None, _trace=False, **_unused):
    global LAST_RESULTS
    from concourse.bass_utils import run_bass_kernel_spmd

    encoder_out = np.ascontiguousarray(np.asarray(encoder_out, np.float32))
    decoder_hidden = np.ascontiguousarray(np.asarray(decoder_hidden, np.float32))
    w_enc = np.ascontiguousarray(np.asarray(W_enc, np.float32))
    b_enc_a = np.asarray(b_enc, np.float32).reshape(1, ATT_DIM)
    w_dec = np.ascontiguousarray(np.asarray(W_dec, np.float32))
    b_dec_a = np.asarray(b_dec, np.float32).reshape(1, ATT_DIM)
    w_full_a = np.asarray(W_full, np.float32).reshape(1, ATT_DIM)
    sel2_all, selT2_all = make_sel_masks()
    sel_flat = np.ascontiguousarray(sel2_all.reshape(2, NT * P))
    selT_flat = np.ascontiguousarray(selT2_all.reshape(P, NT * 2))

    nc = _get_nc()
    in_maps = []
    for c in range(N_CORES):
        sl = slice(c * B, (c + 1) * B)
        in_maps.append({
            "enc": encoder_out[sl].reshape(ROWS, ENC_DIM),
            "dec": decoder_hidden[sl],
            "w_enc": w_enc,
            "b_enc": b_enc_a,
            "w_dec": w_dec,
            "b_dec": b_dec_a,
            "w_full": w_full_a,
            "sel2_all": sel_flat,
            "selT2_all": selT_flat,
        })
    res = run_bass_kernel_spmd(nc, in_maps, core_ids=list(range(N_CORES)),
                               trace=_trace)
    LAST_RESULTS = res
    ctx = np.concatenate([res.results[c]["ctx_out"] for c in range(N_CORES)], 0)
    alpha = np.concatenate([res.results[c]["alpha_out"] for c in range(N_CORES)], 0)
    return ctx, alpha


if __name__ == "__main__":
    nc = _get_nc()
    print("program built and compiled OK")


# revision 21
# speedup vs baseline: 171.6712x; 171.6712x over previous
"""Bahdanau-attention kernel for Trainium2 (Bass/Tile), data-parallel over batch.

Reference computation (per batch b, pixels p=196, enc dim e=2048, att dim a=512):
    att1 = enc @ W_enc                      [B, P, A]
    att2 = dec @ W_dec + b_dec              [B, A]
    s    = tanh(att1 + att2 + b_enc) @ W_full   (+ b_full, irrelevant to softmax)
    alpha = softmax(s, axis=pixels)
    ctx  = sum_p alpha[p] * enc[p]

Sharding: batch 512 -> 8 cores x 64 batches. Weights replicated.

Per-core kernel (single HBM pass over enc):
  - 64*196 = 12544 rows, processed as 98 tiles of 128 rows.
  - per tile: PE-transpose enc -> att1 matmuls (float32r, N=512) into PSUM;
    a "selection" matmul adds att2+biases broadcast per-row; tanh (ScalarE);
    fused mul+reduce (VectorE) for scores; exp (no max subtraction: |s| <=
    sum|W_full| ~ 11.6, exp can't overflow fp32); masked-alpha matmul
    accumulates unnormalized context into persistent PSUM [64, 2048].
  - end: Z = sum exp(s) per batch, alpha = e/Z, ctx = ctx_unnorm/Z.

The row->batch segment masks depend only on shapes; they are precomputed on
the host and shipped as tiny constant inputs (sel2 for the att2-broadcast
matmul, selT2 for masking exp'd scores into per-batch alpha columns).

Measured on trn2 (8 cores, axon): rel err vs fp32 reference ~1.6e-4
(float32r matmul precision); device time ~525-610 us (repeat-slope
measurement / TimelineSim model; PE-bound: att1 stream 334 us + enc
transposes 125 us + context accumulation 84 us + att2 broadcast 21 us).
"""

import sys

for _p in ("/opt/trn_rl_repo",):
    if _p not in sys.path:
        sys.path.insert(0, _p)

from contextlib import ExitStack

import numpy as np

import concourse.bass as bass
import concourse.mybir as mybir
import concourse.tile as tile
from concourse import bacc
from concourse.bass import ds, ts
from concourse.masks import make_identity

FP32 = mybir.dt.float32
FP32R = mybir.dt.float32r

N_CORES = 8
BATCH = 512
NPIX = 196
ENC_DIM = 2048
DEC_DIM = 512
ATT_DIM = 512

B = BATCH // N_CORES          # 64 batches per core
ROWS = B * NPIX               # 12544 rows per core
P = 128                       # partition tile
NT = ROWS // P                # 98 row-tiles
KE = ENC_DIM // P             # 16 contraction blocks for att1
KD = DEC_DIM // P             # 4 contraction blocks for att2
NSL = ENC_DIM // 512          # 4 n-slices of 512 for ctx


def tile_seg(t):
    """(b0, r): tile t rows [0, r) belong to batch b0, rows [r, 128) to b0+1."""
    row0 = t * P
    b0 = row0 // NPIX
    r = min(P, (b0 + 1) * NPIX - row0)
    return b0, r


def make_sel_masks():
    """Compressed segment masks.

    sel2[j, t, m]  = 1.0 iff row 128t+m is in segment j of tile t
    selT2[m, t, j] = same, transposed layout
    """
    sel2 = np.zeros((2, NT, P), np.float32)
    selT2 = np.zeros((P, NT, 2), np.float32)
    for t in range(NT):
        b0, r = tile_seg(t)
        sel2[0, t, 0:r] = 1.0
        selT2[0:r, t, 0] = 1.0
        if r < P:
            sel2[1, t, r:P] = 1.0
            selT2[r:P, t, 1] = 1.0
    return sel2, selT2


def build_program(opts=None):
    o = {"enc_bufs": 4, "encT_bufs": 12, "h_bufs": 3, "scr_bufs": 3,
         "small_bufs": 4, "alt_bufs": 4, "tr_bufs": 4, "a1_bufs": 2,
         "a2p_bufs": 3,
         "no_ctx": False, "no_att1": False, "no_tr": False, "no_score": False,
         "dma_split": 1, "repeat": 1, "dma_tr": 0}
    if opts:
        o.update(opts)
    nc = bacc.Bacc("TRN2", debug=False, num_devices=N_CORES)

    enc_d = nc.dram_tensor("enc", [ROWS, ENC_DIM], FP32R, kind="ExternalInput")
    dec_d = nc.dram_tensor("dec", [B, DEC_DIM], FP32, kind="ExternalInput")
    wenc_d = nc.dram_tensor("w_enc", [ENC_DIM, ATT_DIM], FP32R, kind="ExternalInput")
    benc_d = nc.dram_tensor("b_enc", [1, ATT_DIM], FP32, kind="ExternalInput")
    wdec_d = nc.dram_tensor("w_dec", [DEC_DIM, ATT_DIM], FP32, kind="ExternalInput")
    bdec_d = nc.dram_tensor("b_dec", [1, ATT_DIM], FP32, kind="ExternalInput")
    wfull_d = nc.dram_tensor("w_full", [1, ATT_DIM], FP32, kind="ExternalInput")
    sel_d = nc.dram_tensor("sel2_all", [2, NT * P], FP32R, kind="ExternalInput")
    selT_d = nc.dram_tensor("selT2_all", [P, NT * 2], FP32R, kind="ExternalInput")
    att2b_dram = nc.dram_tensor("att2b_scratch", [B + 1, ATT_DIM], FP32R,
                                kind="Internal")

    ctx_d = nc.dram_tensor("ctx_out", [B, ENC_DIM], FP32, kind="ExternalOutput")
    alpha_d = nc.dram_tensor("alpha_out", [B, NPIX], FP32, kind="ExternalOutput")

    enc_ap = enc_d.ap()
    with tile.TileContext(nc) as tc, ExitStack() as ctx:
        consts = ctx.enter_context(tc.tile_pool(name="consts", bufs=1))

        ident = consts.tile([P, P], FP32)
        make_identity(nc, ident[:])
        ident_r = consts.tile([P, P], FP32R)
        nc.vector.tensor_copy(ident_r[:], ident[:])

        # W_enc block k holds rows [k*128,(k+1)*128), partition = e within block
        wenc_sb = consts.tile([P, KE, ATT_DIM], FP32R)
        nc.sync.dma_start(
            wenc_sb[:], wenc_d.ap().rearrange("(k p) a -> p k a", p=P)
        )
        wdec_sb = consts.tile([P, KD, ATT_DIM], FP32)
        nc.sync.dma_start(
            wdec_sb[:], wdec_d.ap().rearrange("(k p) a -> p k a", p=P)
        )

        selT2_sb = consts.tile([P, NT, 2], FP32R)
        nc.sync.dma_start(selT2_sb[:], selT_d.ap().rearrange("m (t j) -> m t j", t=NT))

        wfull_row = consts.tile([1, ATT_DIM], FP32)
        nc.sync.dma_start(wfull_row[:], wfull_d.ap())
        benc_row = consts.tile([1, ATT_DIM], FP32)
        nc.sync.dma_start(benc_row[:], benc_d.ap())
        bdec_row = consts.tile([1, ATT_DIM], FP32)
        nc.sync.dma_start(bdec_row[:], bdec_d.ap())
        bias_row = consts.tile([1, ATT_DIM], FP32)
        nc.vector.tensor_add(bias_row[:], benc_row[:], bdec_row[:])

        ones_row = consts.tile([1, P], FP32)
        nc.vector.memset(ones_row[:], 1.0)
        zero_row = consts.tile([1, 512], FP32)
        nc.vector.memset(zero_row[:], 0.0)
        zeros_pp = consts.tile([P, P], FP32)
        nc.vector.memset(zeros_pp[:], 0.0)
        zeros_pp_r = consts.tile([P, P], FP32R)
        nc.vector.tensor_copy(zeros_pp_r[:], zeros_pp[:])

        dec_sb = consts.tile([B, DEC_DIM], FP32)
        nc.sync.dma_start(dec_sb[:], dec_d.ap())

        # ---- setup: W_full broadcast [128, 512] and att2b [64, 512] ----
        wfull_sb = consts.tile([P, ATT_DIM], FP32)
        att2b_sb = consts.tile([B, ATT_DIM], FP32R)
        with tc.tile_pool(name="setup_psum", bufs=1, space="PSUM") as spsum:
            wf_ps = spsum.tile([P, ATT_DIM], FP32)
            nc.tensor.matmul(wf_ps[:], ones_row[:, 0:P], wfull_row[:],
                             start=True, stop=True)
            nc.vector.tensor_copy(wfull_sb[:], wf_ps[:])

            # decT: [64, 512] -> 4 x [128, 64]
            decT_sb = consts.tile([P, KD, B], FP32)
            for l in range(KD):
                dT_ps = spsum.tile([P, B], FP32)
                nc.tensor.transpose(dT_ps[:], dec_sb[:, ts(l, P)],
                                    ident[0:B, 0:B])
                nc.vector.tensor_copy(decT_sb[:, l, :], dT_ps[:])

            att2_ps = spsum.tile([B, ATT_DIM], FP32)
            for l in range(KD):
                nc.tensor.matmul(att2_ps[:], decT_sb[:, l, :], wdec_sb[:, l, :],
                                 start=(l == 0), stop=False)
            # += ones[64] x (b_enc + b_dec)
            nc.tensor.matmul(att2_ps[:], ones_row[:, 0:B], bias_row[:],
                             start=False, stop=True)
            nc.vector.tensor_copy(att2b_sb[:], att2_ps[:])
        nc.sync.dma_start(att2b_dram.ap()[0:B, :], att2b_sb[:])
        zrow_r = consts.tile([1, ATT_DIM], FP32R)
        nc.vector.tensor_copy(zrow_r[:], zero_row[:])
        nc.sync.dma_start(att2b_dram.ap()[B:B + 1, :], zrow_r[:])

        # ---- persistent context accumulator, packed [128, 1024] = 2 banks:
        # partition b + 64*h holds e-columns [1024h, 1024h+1024) of batch b
        ctx_ps = None
        if not o["no_ctx"]:
            ctx_psum_pool = ctx.enter_context(
                tc.tile_pool(name="ctx_psum", bufs=1, space="PSUM"))
            ctx_ps = ctx_psum_pool.tile([P, 1024], FP32)
            for n in range(2):
                nc.tensor.matmul(ctx_ps[:, ts(n, 512)], ones_row[:, 0:P],
                                 zero_row[:], start=True, stop=False,
                                 skip_group_check=True)

        # exp'd scores: column t = rows [128t, 128t+128)
        ea_all = consts.tile([P, NT], FP32)

        enc_pool = ctx.enter_context(tc.tile_pool(name="enc", bufs=o["enc_bufs"]))
        encT_pool = ctx.enter_context(tc.tile_pool(name="encT", bufs=o["encT_bufs"]))
        h_pool = ctx.enter_context(tc.tile_pool(name="h", bufs=o["h_bufs"]))
        scratch_pool = ctx.enter_context(tc.tile_pool(name="scratch", bufs=o["scr_bufs"]))
        small_pool = ctx.enter_context(tc.tile_pool(name="small", bufs=o["small_bufs"]))
        alt_pool = ctx.enter_context(tc.tile_pool(name="alt", bufs=o["alt_bufs"]))
        a2p_pool = ctx.enter_context(tc.tile_pool(name="a2p", bufs=o["a2p_bufs"]))
        sel2_pool = ctx.enter_context(tc.tile_pool(name="sel2", bufs=o["a2p_bufs"]))

        with tc.tile_pool(name="tr_psum", bufs=o["tr_bufs"], space="PSUM") as tr_psum, \
             tc.tile_pool(name="att1_psum", bufs=o["a1_bufs"], space="PSUM") as att1_psum:
          for _rep in range(o["repeat"]):
            for t in range(NT):
                row0 = t * P

                enc_t = enc_pool.tile([P, ENC_DIM], FP32R, tag="enc")
                nsp = o["dma_split"]
                for si in range(nsp):
                    w = ENC_DIM // nsp
                    nc.sync.dma_start(enc_t[:, ts(si, w)],
                                      enc_ap[ds(row0, P), ts(si, w)])

                # transpose enc tile: 4 groups of 4x[128,128] -> encT [128,512]
                encT = []
                for g in range(0 if o["no_tr"] else 4):
                    eT = encT_pool.tile([P, 512], FP32R, tag="encT")
                    if g < o["dma_tr"]:
                        # DMA xbar transpose: two 64-out-partition halves/block
                        for j in range(4):
                            k = 4 * g + j
                            nc.sync.dma_start(
                                eT[0:64, ts(j, P)],
                                enc_t[:, ds(k * P, 64)], transpose=True)
                            nc.sync.dma_start(
                                eT[64:P, ts(j, P)],
                                enc_t[:, ds(k * P + 64, 64)], transpose=True)
                        encT.append(eT)
                        continue
                    tr_ps = tr_psum.tile([P, 512], FP32R, tag="tr")
                    for j in range(4):
                        nc.tensor.transpose(tr_ps[:, ts(j, P)],
                                            enc_t[:, ts(4 * g + j, P)], ident_r[:])
                    cs = o.get("copy_split", "alt")
                    if cs == "alt":
                        use_dve = (g % 2 == 0)
                    elif cs == "dve":
                        use_dve = True
                    elif cs == "act":
                        use_dve = False
                    else:  # "3dve"
                        use_dve = (g != 3)
                    if use_dve:
                        nc.vector.tensor_copy(eT[:], tr_ps[:])
                    else:
                        nc.scalar.copy(eT[:], tr_ps[:])
                    encT.append(eT)

                # att1 = encT.T @ W_enc blocks, accumulated over 16 k-blocks
                a1_ps = att1_psum.tile([P, ATT_DIM], FP32, tag="att1")
                if o["no_att1"] or o["no_tr"]:
                    nc.tensor.matmul(a1_ps[:], att2b_sb[0:2, :],
                                     att2b_sb[0:2, :], start=True, stop=True)
                for k in range(0 if (o["no_att1"] or o["no_tr"]) else KE):
                    g, j = divmod(k, 4)
                    nc.tensor.matmul(a1_ps[:], encT[g][:, ts(j, P)],
                                     wenc_sb[:, k, :],
                                     start=(k == 0), stop=False)

                # += sel.T @ att2b : adds att2b[row's batch] to every row
                if not (o["no_att1"] or o["no_tr"]):
                    b0, r = tile_seg(t)
                    a2p = a2p_pool.tile([2, ATT_DIM], FP32R, tag="a2p")
                    nc.sync.dma_start(a2p[:], att2b_dram.ap()[ds(b0, 2), :])
                    s2t = sel2_pool.tile([2, P], FP32R, tag="s2t")
                    nc.sync.dma_start(
                        s2t[:], sel_d.ap().rearrange("j (t m) -> j t m", t=NT)[:, t, :])
                    nc.tensor.matmul(a1_ps[:], s2t[:],
                                     a2p[:], start=False, stop=True)

                # h = tanh(att1), scores = h . W_full
                h = h_pool.tile([P, ATT_DIM], FP32, tag="h")
                nc.scalar.activation(h[:], a1_ps[:],
                                     mybir.ActivationFunctionType.Tanh)
                sc = small_pool.tile([P, 1], FP32, tag="sc")
                if o["no_score"]:
                    nc.vector.tensor_copy(sc[:], h[:, 0:1])
                else:
                    scr = scratch_pool.tile([P, ATT_DIM], FP32, tag="scr")
                    nc.vector.tensor_mul(scr[:], h[:], wfull_sb[:])
                    nc.vector.tensor_reduce(sc[:], scr[:],
                                            axis=mybir.AxisListType.X,
                                            op=mybir.AluOpType.add)

                # ea = exp(scores), collected as column t
                nc.scalar.activation(ea_all[:, ds(t, 1)], sc[:],
                                     mybir.ActivationFunctionType.Exp)

                # alT[:, b] = ea masked to batch b's rows; ctx += alT.T @ enc
                b0t, rt = tile_seg(t)
                ncols = 2 if (rt < P and b0t + 1 < B) else 1
                alTh = []
                for hh in range(2):
                    a = alt_pool.tile([P, P], FP32R, tag=f"alT{hh}")
                    nc.vector.tensor_copy(a[:], zeros_pp_r[:])
                    nc.vector.tensor_scalar_mul(
                        a[:, ds(b0t + 64 * hh, ncols)],
                        selT2_sb[:, t, 0:ncols], ea_all[:, ds(t, 1)])
                    alTh.append(a)
                for n in range(0 if o["no_ctx"] else NSL):
                    nc.tensor.matmul(ctx_ps[:, ts(n % 2, 512)], alTh[n // 2][:],
                                     enc_t[:, ts(n, 512)],
                                     start=False,
                                     stop=(t == NT - 1 and n == NSL - 1),
                                     skip_group_check=True)

        # ---- finalize ----
        e_sb = consts.tile([B, NPIX], FP32)
        with tc.tile_pool(name="fin_psum", bufs=1, space="PSUM") as fpsum:
            eaT_ps = fpsum.tile([NT, P], FP32)
            nc.tensor.transpose(eaT_ps[:], ea_all[:], ident[:])
            eaT_sb = consts.tile([NT, P], FP32)
            nc.vector.tensor_copy(eaT_sb[:], eaT_ps[:])
        # flat row order [NT,128] -> [64,196] reshape via a DRAM bounce
        ea_dram = nc.dram_tensor("ea_scratch", [NT, P], FP32, kind="Internal")
        nc.sync.dma_start(ea_dram.ap(), eaT_sb[:])
        nc.sync.dma_start(e_sb[:],
                          ea_dram.ap().rearrange("t m -> (t m)")
                          .rearrange("(b p) -> b p", b=B))

        z = consts.tile([B, 1], FP32)
        nc.vector.tensor_reduce(z[:], e_sb[:], axis=mybir.AxisListType.X,
                                op=mybir.AluOpType.add)
        rz = consts.tile([B, 1], FP32)
        nc.vector.reciprocal(rz[:], z[:])

        alpha_sb = consts.tile([B, NPIX], FP32)
        nc.vector.tensor_scalar_mul(alpha_sb[:], e_sb[:], rz[:])
        nc.sync.dma_start(alpha_d.ap(), alpha_sb[:])

        ctx_sb = consts.tile([P, 1024], FP32)
        if o["no_ctx"]:
            nc.vector.memset(ctx_sb[:], 0.0)
        else:
            # dup[b, c] = 1 iff c == b or c == b + 64; z2 = dup.T @ z
            dup_sb = consts.tile([B, P], FP32)
            nc.gpsimd.memset(dup_sb[:], 0.0)
            nc.gpsimd.affine_select(
                out=dup_sb[:], in_=dup_sb[:],
                compare_op=mybir.AluOpType.not_equal, fill=1.0, base=0,
                pattern=[[-1, P]], channel_multiplier=1)
            nc.gpsimd.affine_select(
                out=dup_sb[:], in_=dup_sb[:],
                compare_op=mybir.AluOpType.not_equal, fill=1.0, base=64,
                pattern=[[-1, P]], channel_multiplier=1)
            with tc.tile_pool(name="z2_psum", bufs=1, space="PSUM") as zpool:
                z2_ps = zpool.tile([P, 1], FP32)
                nc.tensor.matmul(z2_ps[:], dup_sb[:], z[:],
                                 start=True, stop=True)
                rz2 = consts.tile([P, 1], FP32)
                nc.vector.reciprocal(rz2[:], z2_ps[:])
            nc.vector.tensor_scalar_mul(ctx_sb[:, 0:512], ctx_ps[:, 0:512],
                                        rz2[:])
            nc.scalar.activation(ctx_sb[:, 512:1024], ctx_ps[:, 512:1024],
                                 mybir.ActivationFunctionType.Copy, scale=rz2[:])
        nc.sync.dma_start(ctx_d.ap()[:, 0:1024], ctx_sb[0:B, :])
        nc.sync.dma_start(ctx_d.ap()[:, 1024:2048], ctx_sb[B:P, :])

    nc.compile()
    return nc


_NC_CACHE = None


def _get_nc():
    global _NC_CACHE
    if _NC_CACHE is None:
        _NC_CACHE = build_program()
    return _NC_CACHE


LAST_RESULTS = None


def kernel(encoder_out, decoder_hidden, W_enc, b_enc, W_dec, b_dec, W_full,
           b_full=None, _trace=False, **_unused):
    global LAST_RESULTS
    from concourse.bass_utils import run_bass_kernel_spmd

    encoder_out = np.ascontiguousarray(np.asarray(encoder_out, np.float32))
    decoder_hidden = np.ascontiguousarray(np.asarray(decoder_hidden, np.float32))
    w_enc = np.ascontiguousarray(np.asarray(W_enc, np.float32))
    b_enc_a = np.asarray(b_enc, np.float32).reshape(1, ATT_DIM)
    w_dec = np.ascontiguousarray(np.asarray(W_dec, np.float32))
    b_dec_a = np.asarray(b_dec, np.float32).reshape(1, ATT_DIM)
    w_full_a = np.asarray(W_full, np.float32).reshape(1, ATT_DIM)
    sel2_all, selT2_all = make_sel_masks()
    sel_flat = np.ascontiguousarray(sel2_all.reshape(2, NT * P))
    selT_flat = np.ascontiguousarray(selT2_all.reshape(P, NT * 2))

    nc = _get_nc()
    in_maps = []
    for c in range(N_CORES):
        sl = slice(c * B, (c + 1) * B)
        in_maps.append({
            "enc": encoder_out[sl].reshape(ROWS, ENC_DIM),
            "dec": decoder_hidden[sl],
            "w_enc": w_enc,
            "b_enc": b_enc_a,
            "w_dec": w_dec,
            "b_dec": b_dec_a,
            "w_full": w_full_a,
            "sel2_all": sel_flat,
            "selT2_all": selT_flat,
        })
    res = run_bass_kernel_spmd(nc, in_maps, core_ids=list(range(N_CORES)),
                               trace=_trace)
    LAST_RESULTS = res
    ctx = np.concatenate([res.results[c]["ctx_out"] for c in range(N_CORES)], 0)
    alpha = np.concatenate([res.results[c]["alpha_out"] for c in range(N_CORES)], 0)
    return ctx, alpha


if __name__ == "__main__":
    nc = _get_nc()
    print("program built and compiled OK")
